# revision 2
# baseline (speedup 1.0000x reference)
"""MoE-LoRA Linear kernel for 8 Trainium2 NeuronCores — fp8 DoubleRow version.

Sharding: core c -> (batch b = c//2, out-feature half = c%2).
Each core computes out[b, :, half] = x[b] @ W_half.T + b_half
                                   + SCALING * router-weighted LoRA.

Precision scheme: x, W, lora_A, router_W are quantized to fp8 e4m3 on host.
W/A/rW are pre-scaled by QSCALE=64 so their ~0.02-magnitude entries land in
e4m3's normal range; every PSUM therefore holds 64x the true value and the
final evacuation multiplies by 1/64. The main matmul runs in DoubleRow perf
mode (2 k-subtiles of 128 per instruction, 0.5 cycles/row) for 2x fp8
throughput. LoRA up-projection + bias enter each PSUM group via one K=65
f32r matmul: rows 0-63 of Bta are lora_B[half] (router-weight-scaled on
device), row 64 is 64*b_base matched by a ones-row in the augmented h.

Device layout (per core):
  x8  [4096, 2048] fp8   x[b].T (d-major)
  w8  [4096, 2048] fp8   64*W_base[half].T
  a8  [4096, 64]   fp8   64*lora_A as [d, e*8+r]
  r8  [4096, 8]    fp8   64*router_W.T
  Bta [65, 2048]   f32r  rows 0-63: lora_B[half] as [er, o]; row 64: 64*b_base
  out [2048, 2048] f32   result transposed: [o, t]
"""
import sys

sys.path.insert(0, "/opt/trn_rl_repo")

import ml_dtypes
import numpy as np

import concourse.bass as bass
import concourse.mybir as mybir
import concourse.tile as tile
from concourse import bacc, bass_isa
from concourse.bass_utils import run_bass_kernel_spmd

F32 = mybir.dt.float32
F32R = mybir.dt.float32r
F8 = mybir.dt.float8e4
F8NP = ml_dtypes.float8_e4m3
DOUBLE_ROW = mybir.MatmulPerfMode.DoubleRow

D, T, O_SH, E, R = 4096, 2048, 2048, 8, 8
ER = E * R  # 64
DT = D // 128  # 32 d-tiles
DP = DT // 2  # 16 d-tile pairs (DoubleRow)
NT = 4  # t-chunks of 512
OT = O_SH // 128  # 16 o-tiles
ROUTER_TEMP = 1.0
SCALING = 16.0 / 8.0
QSCALE = 64.0

_nc_cache = []


def build():
    nc = bacc.Bacc(None, target_bir_lowering=False)
    x8 = nc.dram_tensor("x8", [D, T], F8, kind="ExternalInput")
    w8 = nc.dram_tensor("w8", [D, O_SH], F8, kind="ExternalInput")
    a8 = nc.dram_tensor("a8", [D, ER], F8, kind="ExternalInput")
    r8 = nc.dram_tensor("r8", [D, E], F8, kind="ExternalInput")
    Bta = nc.dram_tensor("Bta", [ER + 1, O_SH], F32R, kind="ExternalInput")
    rb = nc.dram_tensor("rb", [E], F32, kind="ExternalInput")
    out = nc.dram_tensor("out", [O_SH, T], F32, kind="ExternalOutput")
    wscratch = nc.dram_tensor("wscratch", [E], F32)

    with tile.TileContext(nc) as tc:
        with (
            tc.tile_pool(name="single", bufs=1) as single,
            tc.tile_pool(name="ev", bufs=4) as evp,
            tc.tile_pool(name="psm", bufs=5, space="PSUM") as psm,
            tc.tile_pool(name="psh", bufs=2, space="PSUM") as psh,
            tc.tile_pool(name="psr", bufs=1, space="PSUM") as psr,
        ):
            xs = single.tile([128, DT, T], F8)
            ws = single.tile([128, DT, O_SH], F8)
            asb = single.tile([128, DT, ER], F8)
            rsb = single.tile([128, DT, E], F8)
            btas = single.tile([ER + 1, O_SH], F32R)
            rbs = single.tile([E, 1], F32)
            haug = single.tile([ER + 1, T], F32R)

            x8r = x8[:].rearrange("(dt p) t -> p dt t", p=128)
            w8r = w8[:].rearrange("(dt p) o -> p dt o", p=128)

            # small tensors first so the router/lora path isn't blocked
            nc.sync.dma_start(asb[:], a8[:].rearrange("(dt p) r -> p dt r", p=128))
            nc.sync.dma_start(rsb[:], r8[:].rearrange("(dt p) e -> p dt e", p=128))
            nc.sync.dma_start(btas[:], Bta[:])
            nc.sync.dma_start(rbs[:], rb[:, None])
            # x chunks interleaved with W quarters
            for q in range(NT):
                tsl = slice(q * 512, (q + 1) * 512)
                nc.sync.dma_start(xs[:, :, tsl], x8r[:, :, tsl])
                nc.sync.dma_start(ws[:, :, tsl], w8r[:, :, tsl])
            nc.vector.memset(haug[ER : ER + 1, :], 1.0)

            # router logits psum accumulates over all (d, t); h per t-chunk
            rtp = psr.tile([E, 512], F32, tag="rt")
            for tch in range(NT):
                tsl = slice(tch * 512, (tch + 1) * 512)
                for dp in range(DP):
                    nc.tensor.matmul(
                        rtp[:],
                        rsb[:, 2 * dp : 2 * dp + 2, :],
                        xs[:, 2 * dp : 2 * dp + 2, tsl],
                        start=(tch == 0 and dp == 0),
                        stop=(tch == NT - 1 and dp == DP - 1),
                        perf_mode=DOUBLE_ROW,
                    )
                hps = psh.tile([ER, 512], F32, tag="h")
                for dp in range(DP):
                    nc.tensor.matmul(
                        hps[:],
                        asb[:, 2 * dp : 2 * dp + 2, :],
                        xs[:, 2 * dp : 2 * dp + 2, tsl],
                        start=(dp == 0),
                        stop=(dp == DP - 1),
                        perf_mode=DOUBLE_ROW,
                    )
                # haug rows hold 64*h (QSCALE from a8); matched by unscaled B rows
                nc.vector.tensor_copy(haug[0:ER, tsl], hps[:])

            # router: logits = rtp summed over t / (T*QSCALE) + rb -> softmax
            lg = single.tile([E, 1], F32)
            nc.vector.reduce_sum(lg[:], rtp[:], axis=mybir.AxisListType.X)
            nc.scalar.activation(
                lg[:], lg[:], mybir.ActivationFunctionType.Copy,
                scale=1.0 / (T * QSCALE * ROUTER_TEMP),
            )
            nc.vector.tensor_tensor(lg[:], lg[:], rbs[:], mybir.AluOpType.add)
            m8 = single.tile([E, 1], F32)
            nc.gpsimd.partition_all_reduce(
                m8[:], lg[:], channels=E, reduce_op=bass_isa.ReduceOp.max
            )
            e8 = single.tile([E, 1], F32)
            nc.vector.tensor_tensor(e8[:], lg[:], m8[:], mybir.AluOpType.subtract)
            nc.scalar.activation(e8[:], e8[:], mybir.ActivationFunctionType.Exp)
            s8 = single.tile([E, 1], F32)
            nc.gpsimd.partition_all_reduce(
                s8[:], e8[:], channels=E, reduce_op=bass_isa.ReduceOp.add
            )
            r8v = single.tile([E, 1], F32)
            nc.vector.reciprocal(r8v[:], s8[:])
            w8v = single.tile([E, 1], F32)
            nc.vector.tensor_tensor(w8v[:], e8[:], r8v[:], mybir.AluOpType.mult)
            nc.vector.tensor_scalar_mul(w8v[:], w8v[:], SCALING)
            nc.sync.dma_start(wscratch[:], w8v[:, 0])
            wexp = single.tile([ER + 1, 1], F32)
            nc.vector.memset(wexp[ER : ER + 1, :], 1.0)
            wsrc = bass.AP(tensor=wscratch, offset=0, ap=[[1, E], [0, R]])
            nc.sync.dma_start(wexp[0:ER, :], wsrc)
            # scale Bta rows by router weight * SCALING (bias row 64 *= 1.0)
            nc.vector.tensor_tensor(
                btas[:], btas[:], wexp[:].to_broadcast([ER + 1, O_SH]),
                mybir.AluOpType.mult,
            )

            # main loop: psum = 64*(x@W.T) via fp8 DoubleRow, then one K=65
            # f32r matmul adds 64*(bias + w*SCALING*lora); evac scales by 1/64
            for tch in range(NT):
                tsl = slice(tch * 512, (tch + 1) * 512)
                for o in range(OT):
                    osl = slice(o * 128, (o + 1) * 128)
                    ps = psm.tile([128, 512], F32, tag="m")
                    for dp in range(DP):
                        nc.tensor.matmul(
                            ps[:],
                            ws[:, 2 * dp : 2 * dp + 2, osl],
                            xs[:, 2 * dp : 2 * dp + 2, tsl],
                            start=(dp == 0),
                            stop=False,
                            perf_mode=DOUBLE_ROW,
                        )
                    nc.tensor.matmul(
                        ps[:], btas[:, osl], haug[:, tsl], start=False, stop=True
                    )
                    ev = evp.tile([128, 512], F32, tag="ev")
                    nc.vector.tensor_scalar_mul(ev[:], ps[:], 1.0 / QSCALE)
                    nc.sync.dma_start(out[osl, tsl], ev[:])
    nc.compile()
    return nc


def _get_nc():
    if not _nc_cache:
        _nc_cache.append(build())
    return _nc_cache[0]


def kernel(x, W_base, b_base, lora_A, lora_B, router_W, router_b):
    x = np.asarray(x, dtype=np.float32)
    W_base = np.asarray(W_base, dtype=np.float32)
    b_base = np.asarray(b_base, dtype=np.float32)
    lora_A = np.asarray(lora_A, dtype=np.float32)
    lora_B = np.asarray(lora_B, dtype=np.float32)
    router_W = np.asarray(router_W, dtype=np.float32)
    router_b = np.asarray(router_b, dtype=np.float32)

    B, S, D_ = x.shape
    O = W_base.shape[0]
    a8_h = np.ascontiguousarray(
        (lora_A.reshape(E * R, D_).T * QSCALE).astype(F8NP)
    )  # [D, 64]
    r8_h = np.ascontiguousarray((router_W.T * QSCALE).astype(F8NP))  # [D, 8]

    x8_hs = [np.ascontiguousarray(x[b].T.astype(F8NP)) for b in range(B)]
    w8_hs = []
    bta_hs = []
    for half in range(2):
        osl = slice(half * O_SH, (half + 1) * O_SH)
        w8_hs.append(
            np.ascontiguousarray((W_base[osl].T * QSCALE).astype(F8NP))
        )
        Bt = np.ascontiguousarray(
            lora_B[:, osl, :].transpose(0, 2, 1).reshape(E * R, O_SH)
        )
        bta_hs.append(
            np.concatenate([Bt, QSCALE * b_base[osl][None, :]], axis=0).astype(
                np.float32
            )
        )

    in_maps = []
    for c in range(8):
        b, half = c // 2, c % 2
        in_maps.append(
            {
                "x8": x8_hs[b],
                "w8": w8_hs[half],
                "a8": a8_h,
                "r8": r8_h,
                "Bta": bta_hs[half],
                "rb": router_b,
            }
        )

    global _last_in_maps
    _last_in_maps = in_maps
    nc = _get_nc()
    res = run_bass_kernel_spmd(nc, in_maps, core_ids=list(range(8)))
    out = np.empty((B, S, O), dtype=np.float32)
    for c in range(8):
        b, half = c // 2, c % 2
        out[b, :, half * O_SH : (half + 1) * O_SH] = res.results[c]["out"].T
    return out


# revision 26
# speedup vs baseline: 1.2246x; 1.2246x over previous
"""MoE-LoRA Linear kernel for 8 Trainium2 NeuronCores — split-K fp8/bf16 hybrid.

Sharding: core c -> (batch b = c//2, out-feature half = c%2).
Each core computes out[b, :, half] = x[b] @ W_half.T + b_half
                                   + SCALING * router-weighted LoRA.

Precision scheme (split along the contraction dim D=4096):
  - lower DLT*128 = 1792 of d: fp8 e4m3, DoubleRow perf mode (2 k-subtiles
    per instruction, 0.5 cycles/row = 2x f32r throughput). Dual-fp8
    LDWEIGHTS allows at most 128 weight elements/partition, so DoubleRow
    output is limited to 64 partitions: each 128-row psum tile takes its
    fp8 contribution as two 64-row halves from per-o64 weight tiles whose
    [2, 64] k-pair slices are contiguous in SBUF.
  - upper DUT*128 = 2304 of d: bf16 (1 cycle/row), full 128-col tiles.
  Quantization noise comes only from the fp8 fraction: absmax-rel ~1.7e-2
  (gate 2e-2). W/lora_A/router_W are pre-scaled by 64 in BOTH halves so
  every psum holds 64x the true value; evacuation multiplies by 1/64.

Router: logits = (fp8 DoubleRow psum accumulated over lower d) + (rows
64:72 of the combined bf16 lora_A+router_W psum), reduced over t ->
softmax on gpsimd; weights broadcast e-major via a dram round-trip on the
gpsimd DMA queue and applied in-place to haug rows. Bta stays read-only
(host folds SCALING into B rows, 64x into the bias row); LoRA + bias
enter each psum group via one K=65 f32r matmul.
"""
import sys

sys.path.insert(0, "/opt/trn_rl_repo")

import ml_dtypes
import numpy as np

import concourse.bass as bass
import concourse.mybir as mybir
import concourse.tile as tile
from concourse import bacc, bass_isa
from concourse.bass_utils import run_bass_kernel_spmd

F32 = mybir.dt.float32
F32R = mybir.dt.float32r
F8 = mybir.dt.float8e4
BF16 = mybir.dt.bfloat16
F8NP = ml_dtypes.float8_e4m3
BFNP = ml_dtypes.bfloat16
DOUBLE_ROW = mybir.MatmulPerfMode.DoubleRow

D, T, O_SH, E, R = 4096, 2048, 2048, 8, 8
ER = E * R  # 64
DLT = 14  # lower d-tiles (fp8): K = 1792
DPL = DLT // 2  # 7 d-tile pairs
DUT = 32 - DLT  # upper d-tiles (bf16): K = 2304
DL, DU = DLT * 128, DUT * 128
NT = 4  # t-chunks of 512
OT = O_SH // 128  # 16 o-tiles
EP = 16  # router fp8 cols padded to 16 (dual-fp8 LW needs 16B-aligned pair step)
ROUTER_TEMP = 1.0
SCALING = 16.0 / 8.0
QSCALE = 64.0

_nc_cache = []


def build():
    nc = bacc.Bacc(None, target_bir_lowering=False)
    x8d = nc.dram_tensor("x8d", [DL, T], F8, kind="ExternalInput")
    xbd = nc.dram_tensor("xbd", [DU, T], BF16, kind="ExternalInput")
    # fp8 W per o64-tile: [o64, p, dlt, 64]; bf16 W per o128-tile: [o, p, dut, 128]
    w8d = nc.dram_tensor("w8d", [2 * OT, 128, DLT, 64], F8, kind="ExternalInput")
    wbd = nc.dram_tensor("wbd", [OT, 128, DUT, 128], BF16, kind="ExternalInput")
    a8d = nc.dram_tensor("a8d", [DL, ER], F8, kind="ExternalInput")
    r8d = nc.dram_tensor("r8d", [DL, EP], F8, kind="ExternalInput")
    abd = nc.dram_tensor("abd", [DU, ER], BF16, kind="ExternalInput")
    rbd16 = nc.dram_tensor("rbd16", [DU, E], BF16, kind="ExternalInput")
    Bta = nc.dram_tensor("Bta", [ER + 1, O_SH], F32R, kind="ExternalInput")
    rb = nc.dram_tensor("rb", [E], F32, kind="ExternalInput")
    out = nc.dram_tensor("out", [O_SH, T], F32, kind="ExternalOutput")
    wscratch = nc.dram_tensor("wscratch", [E], F32)

    with tile.TileContext(nc) as tc:
        with (
            tc.tile_pool(name="single", bufs=1) as single,
            tc.tile_pool(name="w8p", bufs=2 * OT) as w8p,
            tc.tile_pool(name="wbp", bufs=3) as wbp,
            tc.tile_pool(name="ev", bufs=4) as evp,
            tc.tile_pool(name="psm", bufs=3, space="PSUM") as psm,
            tc.tile_pool(name="ps64", bufs=4, space="PSUM") as ps64,
            tc.tile_pool(name="psr", bufs=1, space="PSUM") as psr,
        ):
            x8s = single.tile([128, DLT, T], F8)
            xbs = single.tile([128, DUT, T], BF16)
            a8s = single.tile([128, DLT, ER], F8)
            r8s = single.tile([128, DLT, EP], F8)
            ab16 = single.tile([128, DUT, ER], BF16)
            rb16 = single.tile([128, DUT, E], BF16)
            btas = single.tile([ER + 1, O_SH], F32R)
            rbs = single.tile([E, 1], F32)
            haug = single.tile([ER + 1, T], F32R)

            # gpsimd queue: smalls + all fp8-side tensors (small, needed early)
            nc.gpsimd.dma_start(a8s[:], a8d[:].rearrange("(k p) r -> p k r", p=128))
            nc.gpsimd.dma_start(r8s[:], r8d[:].rearrange("(k p) e -> p k e", p=128))
            nc.gpsimd.dma_start(btas[:], Bta[:])
            nc.gpsimd.dma_start(rbs[:], rb[:, None])
            nc.gpsimd.dma_start(x8s[:], x8d[:].rearrange("(k p) t -> p k t", p=128))
            w8ts = []
            for j in range(2 * OT):
                w8t = w8p.tile([128, DLT, 64], F8, tag="w8", name=f"w8_{j}")
                nc.gpsimd.dma_start(w8t[:], w8d[j])
                w8ts.append(w8t)
            # sync queue: bf16 x chunks (bulk), then per-o bf16 W in the loop
            xbr = xbd[:].rearrange("(k p) t -> p k t", p=128)
            for q in range(NT):
                tsl = slice(q * 512, (q + 1) * 512)
                nc.sync.dma_start(xbs[:, :, tsl], xbr[:, :, tsl])
            nc.scalar.dma_start(ab16[:], abd[:].rearrange("(k p) r -> p k r", p=128))
            nc.scalar.dma_start(rb16[:], rbd16[:].rearrange("(k p) e -> p k e", p=128))
            nc.vector.memset(haug[ER : ER + 1, :].bitcast(F32), 1.0)

            # aux: h (lora down-proj) + router logits; each psum group mixes
            # fp8 DoubleRow (lower d) and bf16 (upper d) instructions
            rtp = psr.tile([EP, 512], F32, tag="rt")
            for tch in range(NT):
                tsl = slice(tch * 512, (tch + 1) * 512)
                for dp in range(DPL):
                    nc.tensor.matmul(
                        rtp[:],
                        r8s[:, 2 * dp : 2 * dp + 2, :],
                        x8s[:, 2 * dp : 2 * dp + 2, tsl],
                        start=(tch == 0 and dp == 0),
                        stop=False,
                        perf_mode=DOUBLE_ROW,
                    )
                for du in range(DUT):
                    nc.tensor.matmul(
                        rtp[0:E, :],
                        rb16[:, du, :],
                        xbs[:, du, tsl],
                        start=False,
                        stop=(tch == NT - 1 and du == DUT - 1),
                    )
                hps = ps64.tile([ER, 512], F32, tag="p64")
                for dp in range(DPL):
                    nc.tensor.matmul(
                        hps[:],
                        a8s[:, 2 * dp : 2 * dp + 2, :],
                        x8s[:, 2 * dp : 2 * dp + 2, tsl],
                        start=(dp == 0),
                        stop=False,
                        perf_mode=DOUBLE_ROW,
                    )
                for du in range(DUT):
                    nc.tensor.matmul(
                        hps[:],
                        ab16[:, du, :],
                        xbs[:, du, tsl],
                        start=False,
                        stop=(du == DUT - 1),
                    )
                # haug rows hold 64*h; router weight applied later in-place
                nc.vector.tensor_copy(haug[0:ER, tsl], hps[:])

            # router: logits = rtp summed over t / (T*64) + rb -> softmax
            lg = single.tile([E, 1], F32)
            nc.vector.reduce_sum(lg[:], rtp[0:E, :], axis=mybir.AxisListType.X)
            nc.scalar.activation(
                lg[:], lg[:], mybir.ActivationFunctionType.Copy,
                scale=1.0 / (T * QSCALE * ROUTER_TEMP),
            )
            nc.vector.tensor_tensor(lg[:], lg[:], rbs[:], mybir.AluOpType.add)
            m8 = single.tile([E, 1], F32)
            nc.gpsimd.partition_all_reduce(
                m8[:], lg[:], channels=E, reduce_op=bass_isa.ReduceOp.max
            )
            e8 = single.tile([E, 1], F32)
            nc.vector.tensor_tensor(e8[:], lg[:], m8[:], mybir.AluOpType.subtract)
            nc.scalar.activation(e8[:], e8[:], mybir.ActivationFunctionType.Exp)
            s8 = single.tile([E, 1], F32)
            nc.gpsimd.partition_all_reduce(
                s8[:], e8[:], channels=E, reduce_op=bass_isa.ReduceOp.add
            )
            r8v = single.tile([E, 1], F32)
            nc.vector.reciprocal(r8v[:], s8[:])
            w8v = single.tile([E, 1], F32)
            nc.vector.tensor_tensor(w8v[:], e8[:], r8v[:], mybir.AluOpType.mult)
            # broadcast w[e] -> 64 e-major rows via dram round-trip (gpsimd queue)
            nc.gpsimd.dma_start(wscratch[:], w8v[:, 0])
            wexp = single.tile([ER, 1], F32)
            wsrc = bass.AP(tensor=wscratch, offset=0, ap=[[1, E], [0, R]])
            nc.gpsimd.dma_start(wexp[:], wsrc)
            # apply router weights in-place to haug rows (bias row untouched)
            nc.vector.tensor_tensor(
                haug[0:ER, :], haug[0:ER, :], wexp[:].to_broadcast([ER, T]),
                mybir.AluOpType.mult,
            )

            # main loop: per (o128, tch): a [128,512] psum takes the bf16 pass
            # (upper d) + K=65 f32r lora/bias matmul; the fp8 DoubleRow pass
            # (lower d) must write partition-0 psums, so each o64 half gets
            # its own [64,512] psum whose evacuation is DMA-accumulated into
            # the already-written out rows. All evacs scale by 1/64.
            for o in range(OT):
                osl = slice(o * 128, (o + 1) * 128)
                wbt = wbp.tile([128, DUT, 128], BF16, tag="wb")
                nc.sync.dma_start(wbt[:], wbd[o])
                for tch in range(NT):
                    tsl = slice(tch * 512, (tch + 1) * 512)
                    ps = psm.tile([128, 512], F32, tag="m")
                    for du in range(DUT):
                        nc.tensor.matmul(
                            ps[:],
                            wbt[:, du, :],
                            xbs[:, du, tsl],
                            start=(du == 0),
                            stop=False,
                        )
                    p8s = [
                        ps64.tile([64, 512], F32, tag="p64", name=f"p8_{o}_{tch}_{j}")
                        for j in range(2)
                    ]
                    for j2 in range(2):
                        for dp in range(DPL):
                            nc.tensor.matmul(
                                p8s[j2][:],
                                w8ts[2 * o + j2][:, 2 * dp : 2 * dp + 2, :],
                                x8s[:, 2 * dp : 2 * dp + 2, tsl],
                                start=(dp == 0),
                                stop=(dp == DPL - 1),
                                perf_mode=DOUBLE_ROW,
                            )
                    nc.tensor.matmul(
                        ps[:], btas[:, osl], haug[:, tsl], start=False, stop=True
                    )
                    ev = evp.tile([128, 512], F32, tag="ev")
                    nc.vector.tensor_scalar_mul(ev[:], ps[:], 1.0 / QSCALE)
                    nc.scalar.dma_start(out[osl, tsl], ev[:])
                    for j2 in range(2):
                        ev8 = evp.tile([64, 512], F32, tag="ev8")
                        nc.vector.tensor_scalar_mul(ev8[:], p8s[j2][:], 1.0 / QSCALE)
                        nc.gpsimd.dma_start(
                            out[o * 128 + j2 * 64 : o * 128 + (j2 + 1) * 64, tsl],
                            ev8[:],
                            accum_op=mybir.AluOpType.add,
                        )
    nc.compile()
    return nc


def _get_nc():
    if not _nc_cache:
        _nc_cache.append(build())
    return _nc_cache[0]


def kernel(x, W_base, b_base, lora_A, lora_B, router_W, router_b):
    x = np.asarray(x, dtype=np.float32)
    W_base = np.asarray(W_base, dtype=np.float32)
    b_base = np.asarray(b_base, dtype=np.float32)
    lora_A = np.asarray(lora_A, dtype=np.float32)
    lora_B = np.asarray(lora_B, dtype=np.float32)
    router_W = np.asarray(router_W, dtype=np.float32)
    router_b = np.asarray(router_b, dtype=np.float32)

    B, S, D_ = x.shape
    O = W_base.shape[0]
    AT = lora_A.reshape(E * R, D_).T * QSCALE  # [D, 64], 64x
    rT = router_W.T * QSCALE  # [D, 8], 64x
    a8_h = np.ascontiguousarray(AT[:DL].astype(F8NP))
    r8_h = np.ascontiguousarray(
        np.concatenate([rT[:DL], np.zeros((DL, EP - E), np.float32)], axis=1).astype(F8NP)
    )
    ab_h = np.ascontiguousarray(AT[DL:].astype(BFNP))
    rb16_h = np.ascontiguousarray(rT[DL:].astype(BFNP))

    x8_hs = [np.ascontiguousarray(x[b, :, :DL].T.astype(F8NP)) for b in range(B)]
    xb_hs = [np.ascontiguousarray(x[b, :, DL:].T.astype(BFNP)) for b in range(B)]
    w8_hs, wb_hs, bta_hs = [], [], []
    for half in range(2):
        osl = slice(half * O_SH, (half + 1) * O_SH)
        Ws = W_base[osl].T * QSCALE  # [D, O_SH], 64x
        # fp8 lower: o64-major [2*OT, 128(p), DLT, 64] with d = k*128 + p
        w8_hs.append(
            np.ascontiguousarray(
                Ws[:DL]
                .astype(F8NP)
                .reshape(DLT, 128, 2 * OT, 64)
                .transpose(2, 1, 0, 3)
            )
        )
        # bf16 upper: o128-major [OT, 128(p), DUT, 128]
        wb_hs.append(
            np.ascontiguousarray(
                Ws[DL:]
                .astype(BFNP)
                .reshape(DUT, 128, OT, 128)
                .transpose(2, 1, 0, 3)
            )
        )
        Bt = np.ascontiguousarray(
            lora_B[:, osl, :].transpose(0, 2, 1).reshape(E * R, O_SH)
        ) * np.float32(SCALING)
        bta_hs.append(
            np.concatenate([Bt, QSCALE * b_base[osl][None, :]], axis=0).astype(
                np.float32
            )
        )

    in_maps = []
    for c in range(8):
        b, half = c // 2, c % 2
        in_maps.append(
            {
                "x8d": x8_hs[b],
                "xbd": xb_hs[b],
                "w8d": w8_hs[half],
                "wbd": wb_hs[half],
                "a8d": a8_h,
                "r8d": r8_h,
                "abd": ab_h,
                "rbd16": rb16_h,
                "Bta": bta_hs[half],
                "rb": router_b,
            }
        )

    global _last_in_maps
    _last_in_maps = in_maps
    nc = _get_nc()
    res = run_bass_kernel_spmd(nc, in_maps, core_ids=list(range(8)))
    out = np.empty((B, S, O), dtype=np.float32)
    for c in range(8):
        b, half = c // 2, c % 2
        out[b, :, half * O_SH : (half + 1) * O_SH] = res.results[c]["out"].T
    return out
